# revision 32
# baseline (speedup 1.0000x reference)
# Trainium2 Bass kernel for nn_AStarPathfinder — staging-free precompute.
#
# Pipeline (per core j of NC, SPMD identical instruction stream):
#   - host passes a channel-major zero-padded feature slice for a column band
#   - device computes sobel/avg-pool stencils in layout A ([C partitions,
#     pixels free], full-width contiguous APs), channel contractions via
#     stationary-operand PE matmuls with CR=128-row chunks (outputs
#     pixel-partitioned, block-major: slot = xh*FW + c),
#   - row-shifted field variants (rs(x+-1), dot(x+1)) via 10 small PE
#     shift-matmuls on the evacuated fields (su/sd/seam stationaries) —
#     NO DRAM staging round-trips,
#   - the BF cost table e_t [HP, 9*YLEN] is assembled directly in SBUF:
#     4 unshifted cost dirs + 4 shifted variants (which double as the
#     mirror cost channels 4..7 of the output),
#   - 256 Jacobi Bellman-Ford sweeps, column-sharded with ghost zones,
#     halo refresh via AllGather every T sweeps, wavefront restriction
#     for the first HP-1 sweeps when start is on row 0.
#   - cfg["REPS"] repeats the whole pipeline in-NEFF (used by test.py to
#     measure device time by amplification differencing).
#
# NOTE (hardware constraint): compute-engine SBUF/PSUM accesses must start at
# a partition base that is a multiple of 32 — arbitrary +-1 partition-shifted
# reads are rejected, so all row shifts go through PE shift-matmuls.
import ml_dtypes
import numpy as np

BF16 = np.dtype(ml_dtypes.bfloat16)
BIGF = 1e9
DIRS = [(-1, -1), (-1, 0), (-1, 1), (0, -1), (0, 1), (1, -1), (1, 0), (1, 1)]
F32 = np.float32


def make_cfg(H=256, W=256, C=128, NC=8, K=256, T=10):
    assert C == 128
    cfg = {}
    cfg["H"], cfg["W"], cfg["C"], cfg["NC"], cfg["K"], cfg["T"] = H, W, C, NC, K, T
    cfg["HP"] = H // 2          # partitions for fields / BF (x folded in 2 blocks)
    cfg["XH"] = 2               # number of x blocks
    cfg["BAND"] = W // NC       # owned cols per core
    assert cfg["BAND"] >= 2 * T, "halo exchange strips must fit in owned band"
    cfg["HELD"] = cfg["BAND"] + 2 * T
    cfg["SLICE_W"] = cfg["HELD"] + 4
    cfg["FW"] = cfg["HELD"] + 2  # cost domain cols (held +- 1)
    cfg["XP"] = H + 2
    cfg["TOFF"] = 4              # free-dim guard pad for +-1 col reads
    cfg["FA_FREE"] = cfg["XP"] * cfg["SLICE_W"]
    cfg["CR"] = cfg["HP"]        # stationary-chunk rows (one x-block per chunk)
    assert H % cfg["CR"] == 0
    cfg["NCHK"] = H // cfg["CR"]
    cfg["YB"] = cfg["HELD"] + 2  # BF per-block slot span (== FW)
    cfg["YLEN"] = 2 * cfg["YB"]
    return cfg


# ---------------------------------------------------------------- program ---

def build_program(cfg):
    import concourse.bass as bass
    import concourse.bacc as bacc
    import concourse.mybir as mybir
    import concourse.tile as tile
    from concourse.ap import AP

    H, W, C, NC, K, T = (cfg[k] for k in ("H", "W", "C", "NC", "K", "T"))
    HP, XH, BAND, HELD = (cfg[k] for k in ("HP", "XH", "BAND", "HELD"))
    SLICE_W, FW, XP, YB, YLEN = (
        cfg[k] for k in ("SLICE_W", "FW", "XP", "YB", "YLEN"))
    FA_FREE, TOFF = cfg["FA_FREE"], cfg["TOFF"]
    CR, NCHK = cfg["CR"], cfg["NCHK"]
    dt = mybir.dt.float32
    dtb = mybir.dt.bfloat16
    Alu = mybir.AluOpType
    Act = mybir.ActivationFunctionType

    nc = bacc.Bacc("TRN2", target_bir_lowering=False, debug=False, num_devices=NC)

    # ---- external inputs (per core) ----
    fa_in = nc.dram_tensor("fa", [C, FA_FREE + 8], dt, kind="ExternalInput")
    w1_in = nc.dram_tensor("w1", [C, 32], dt, kind="ExternalInput")
    b1_in = nc.dram_tensor("b1", [32, 1], dt, kind="ExternalInput")
    w2_in = nc.dram_tensor("w2", [32, 1], dt, kind="ExternalInput")
    elf_in = nc.dram_tensor("elf", [64, 1], dt, kind="ExternalInput")
    hsc_in = nc.dram_tensor("hsc", [HP, 8], dt, kind="ExternalInput")  # d,g,b,info,b2
    msk_in = nc.dram_tensor("msk", [HP, 8 * FW * XH], dt, kind="ExternalInput")
    dinit_in = nc.dram_tensor("dinit", [HP, YLEN], dt, kind="ExternalInput")
    shm_in = nc.dram_tensor("shm", [HP, 5 * HP], dt, kind="ExternalInput")  # su|useam|sd|dseam
    out_t = nc.dram_tensor("out", [HP, BAND * XH * 10], dt, kind="ExternalOutput")

    # ---- internal DRAM (halo exchange bounce buffers) ----
    bounce_in = nc.dram_tensor("bounce_in", [HP, 4 * T], dt)
    bounce_out = nc.dram_tensor("bounce_out", [NC * HP, 4 * T], dt, addr_space="Shared")

    CHR = (CR + 2)  # x-chunk temp rows incl halo
    FXX = FW * XH
    # field group indices in ps_all / flds
    G_GEO, G_NALL, G_ASQ, G_VAR, G_WL, G_DOT = 0, 1, 2, 3, 4, 5  # dots: 5..8

    def fap(t, off, dims):
        """AP over tile/dram handle t with explicit free dims (list of [step,count])."""
        base = t[:] if not isinstance(t, AP) else t
        return AP(base.tensor, base.offset + off, [base.ap[0]] + dims)

    with tile.TileContext(nc) as tc:
        with tc.tile_pool(name="persist", bufs=1) as pp, \
             tc.tile_pool(name="chunk", bufs=1) as cp, \
             tc.tile_pool(name="small", bufs=1) as sp, \
             tc.tile_pool(name="psum", bufs=1, space="PSUM") as psp, \
             tc.tile_pool(name="psbf", bufs=2, space="PSUM") as psbf:

            def one_pass():
                # ---------------- load ----------------
                fA = pp.tile([C, FA_FREE + 8], dt, tag="fA")
                FQ = (FA_FREE + 8) // 4
                for q in range(4):
                    n = FQ if q < 3 else FA_FREE + 8 - 3 * FQ
                    nc.sync.dma_start(fap(fA[:], q * FQ, [[1, n]]),
                                      fap(fa_in[:], q * FQ, [[1, n]]))
                w1t = sp.tile([C, 32], dt, tag="w1")
                nc.sync.dma_start(w1t[:], w1_in[:])
                b1t = sp.tile([32, 1], dt, tag="b1")
                nc.sync.dma_start(b1t[:], b1_in[:])
                w2t = sp.tile([32, 1], dt, tag="w2")
                nc.sync.dma_start(w2t[:], w2_in[:])
                elft = sp.tile([64, 1], dt, tag="elf")
                nc.sync.dma_start(elft[:], elf_in[:])
                hsct = sp.tile([HP, 8], dt, tag="hsc")
                nc.sync.dma_start(hsct[:], hsc_in[:])
                mskt = sp.tile([HP, 8 * FXX], dt, tag="msk")
                nc.sync.dma_start(mskt[:], msk_in[:])
                shmt = sp.tile([HP, 5 * HP], dt, tag="shm")
                nc.sync.dma_start(shmt[:], shm_in[:])

                # reduce vectors: 0: 1/C, 1: ones, 2: ones_lo(0:64), 3: ones_hi(64:128)
                vec = sp.tile([C, 4], dt, tag="vec")
                nc.vector.memset(vec[:, 0:1], 1.0 / C)
                nc.vector.memset(vec[:, 1:2], 1.0)
                nc.vector.memset(vec[:, 2:4], 0.0)
                nc.vector.memset(vec[0:64, 2:3], 1.0)
                nc.vector.memset(vec[64:128, 3:4], 1.0)
                vecb = sp.tile([C, 4], dtb, tag="vecb")
                nc.vector.memset(vecb[:, 0:1], 1.0 / C)
                nc.vector.memset(vecb[:, 1:2], 1.0)
                nc.vector.memset(vecb[:, 2:4], 0.0)
                nc.vector.memset(vecb[0:64, 2:3], 1.0)
                nc.vector.memset(vecb[64:128, 3:4], 1.0)

                # field tiles
                flds = sp.tile([HP, 9 * FXX], dt, tag="flds")
                flds2 = sp.tile([HP, 5 * FXX], dt, tag="flds2")

                # psum accumulators
                ps_all = psp.tile([HP, 9 * FXX + 16], dt, tag="psall")

                # Bootstrap LDWEIGHTS fences for constants (see fence_mm below).
                from concourse.bass import _add_dep_helper as _adh

                def boot_fence(lhsT, rhs_t, n, step):
                    outap = fap(ps_all[0:1, :], 9 * FXX, [[1, n]])
                    return nc.tensor.matmul(outap, lhsT,
                                            fap(rhs_t[:], 0, [[step, n]]),
                                            start=True, stop=True).ins

                f0 = boot_fence(vec[:, 0:1], vec, 4, 1)
                f0b = boot_fence(vecb[:, 1:2], vecb, 4, 1)
                _adh(f0b, f0, sync=False, reason="ldw fence chain")
                f_w1 = boot_fence(vec[:, 1:2], w1t, 8, 4)
                _adh(f_w1, f0, sync=False, reason="ldw fence chain")
                f_shm = boot_fence(vec[0:HP, 1:2], shmt, 8, (5 * HP) // 8)
                _adh(f_shm, f0, sync=False, reason="ldw fence chain")

                # ---------------- per-chunk stencils + channel contractions ---
                skip_chunks = bool(cfg.get("skip_chunks"))
                skip_stats = bool(cfg.get("skip_stats"))
                if skip_chunks or skip_stats:
                    nc.vector.memset(ps_all[:], 0.0)
                for ci in range(0 if skip_chunks else NCHK):
                    x0 = ci * CR
                    xh = ci
                    tA = cp.tile([C, CHR * SLICE_W + 8], dt, tag="tA")
                    tB = cp.tile([C, CHR * SLICE_W + 8], dt, tag="tB")
                    tC = cp.tile([C, CHR * SLICE_W + 8], dt, tag="tC")
                    hrel = cp.tile([32, (CR // 2) * SLICE_W], dt, tag="hrel")

                    def fa_c(dx=0, dy=0, rows=(0, CHR), parts=None):
                        # contiguous full-width fA view at (x0+row+dx, col+dy)
                        off = TOFF + (x0 + rows[0] + dx) * SLICE_W + dy
                        n = (rows[1] - rows[0]) * SLICE_W
                        a = fA[:] if parts is None else fA[parts[0]:parts[1], :]
                        return fap(a, off, [[1, n]])

                    def t_c(t, dx=0, dy=0, rows=(0, CHR), parts=None):
                        off = TOFF + (rows[0] + dx) * SLICE_W + dy
                        n = (rows[1] - rows[0]) * SLICE_W
                        a = t[:] if parts is None else t[parts[0]:parts[1], :]
                        return fap(a, off, [[1, n]])

                    RIN = (0, CHR)          # all chunk rows (incl halo)
                    RMID = (1, CHR - 1)     # chunk rows = x in [x0, x0+CR)

                    # LDWEIGHTS RAW fence: PE pulls LDWEIGHTS ahead of in-flight
                    # MATMULs; a tiny fence matmul whose MOVING operand spans the
                    # dataset blocks later LDWEIGHTS until the data is written.
                    def fence_mm(data_t, parts=(0, C), flat=False, vt=None):
                        a = data_t[parts[0]:parts[1], :]
                        if flat:
                            n = a.shape[1]
                            step = max(1, (n - 1) // 15)
                            cnt = min(16, 1 + (n - 1) // step)
                            rhs = fap(a, 0, [[step, cnt]])
                        else:
                            nrow = min(8, CHR - 2)
                            rstep = max(1, (CHR - 2) // nrow)
                            rhs = fap(a, TOFF + SLICE_W + 1,
                                      [[SLICE_W * rstep, nrow], [1, 2]])
                            cnt = nrow * 2
                        if vt is None:
                            vt = vec
                        lhsT = vt[parts[0]:parts[1], 1:2]
                        outap = fap(ps_all[0:1, :], 9 * FXX, [[1, cnt]])
                        mm = nc.tensor.matmul(outap, lhsT, rhs, start=True, stop=True)
                        return mm.ins

                    def stat_mm(group, data_t, c, rhs_ap, parts=(0, C), ncol=1,
                                halo_off=None, fence=None, rows=None):
                        r0, r1 = (0, CR) if rows is None else rows
                        off = (TOFF + SLICE_W if halo_off is None else halo_off)
                        lhsT = fap(data_t[parts[0]:parts[1], :],
                                   off + c + 1, [[SLICE_W, r1 - r0]])
                        base = ps_all[r0:r1, :]
                        outap = AP(base.tensor,
                                   base.offset + group * FXX + xh * FW + c,
                                   [base.ap[0], [FXX, ncol]])
                        mm = nc.tensor.matmul(outap, lhsT, rhs_ap, start=True, stop=True)
                        if fence is not None:
                            _adh(mm.ins, fence, sync=False,
                                 reason="ldweights raw fence")

                    # ---- sobel: gx = smooth_x(f) diff_y ; gy = smooth_y(f) diff_x
                    nc.vector.tensor_tensor(t_c(tA, rows=RMID), fa_c(-1, 0, RMID),
                                            fa_c(+1, 0, RMID), Alu.add)
                    nc.vector.scalar_tensor_tensor(
                        t_c(tA, rows=RMID), fa_c(0, 0, RMID), 2.0,
                        t_c(tA, rows=RMID), Alu.mult, Alu.add)  # sx in tA
                    nc.vector.tensor_tensor(
                        t_c(tB, rows=RMID), t_c(tA, 0, +1, RMID),
                        t_c(tA, 0, -1, RMID), Alu.subtract)  # gx in tB
                    nc.vector.tensor_tensor(t_c(tA), fa_c(0, -1),
                                            fa_c(0, +1), Alu.add)
                    nc.vector.scalar_tensor_tensor(
                        t_c(tA), fa_c(0, 0), 2.0, t_c(tA), Alu.mult, Alu.add)  # sy
                    nc.vector.tensor_tensor(
                        t_c(tC, rows=RMID), t_c(tA, +1, 0, RMID),
                        t_c(tA, -1, 0, RMID), Alu.subtract)  # gy in tC
                    nc.scalar.activation(
                        t_c(tA, rows=RMID), t_c(tB, rows=RMID), Act.Square)  # gx^2
                    nc.vector.tensor_tensor(
                        t_c(tB, rows=RMID), t_c(tC, rows=RMID),
                        t_c(tC, rows=RMID), Alu.mult)  # gy^2 in tB
                    nc.vector.tensor_tensor(
                        t_c(tA, rows=RMID), t_c(tA, rows=RMID),
                        t_c(tB, rows=RMID), Alu.add)   # mag^2 in tA
                    nc.scalar.activation(
                        t_c(tA, rows=RMID), t_c(tA, rows=RMID), Act.Sqrt)  # mag
                    FWs = 0 if skip_stats else FW
                    HC0, HC1 = T + 1, T + 1 + BAND  # heuristic-only col range
                    if skip_stats:
                        HC0, HC1 = 0, 0
                    fng = fence_mm(tA)
                    for c in range(HC0, HC1):
                        stat_mm(G_GEO, tA, c, vec[:, 0:1], fence=fng)  # geo

                    # ---- f^2 -> nall (bf16 tile: halves LDWEIGHTS) ----
                    tD16 = cp.tile([C, CHR * SLICE_W + 8], dtb, tag="tD16")
                    nc.scalar.activation(
                        t_c(tD16, rows=RMID), fa_c(0, 0, RMID), Act.Square)
                    fnn = fence_mm(tD16, vt=vecb)
                    for c in range(FWs):
                        stat_mm(G_NALL, tD16, c, vecb[:, 1:2], fence=fnn)

                    # ---- pools on hf (parts 64:128) + absorption diff (0:64) --
                    # end state: tC[0:64] = diff^2 ; tC[64:128] = poolsq/9 - pool^2/81
                    P64 = (64, 128)
                    P0_64 = (0, 64)
                    # pool(hf): y-pass in tA[64:], x-pass into tC[64:]
                    nc.vector.tensor_tensor(t_c(tA, parts=P64),
                                            fa_c(0, -1, RIN, P64),
                                            fa_c(0, +1, RIN, P64), Alu.add)
                    nc.vector.scalar_tensor_tensor(
                        t_c(tA, parts=P64), fa_c(0, 0, RIN, P64), 1.0,
                        t_c(tA, parts=P64), Alu.mult, Alu.add)  # p3y(hf)
                    nc.vector.tensor_tensor(
                        t_c(tC, 0, 0, RMID, P64), t_c(tA, +1, 0, RMID, P64),
                        t_c(tA, -1, 0, RMID, P64), Alu.add)
                    nc.vector.scalar_tensor_tensor(
                        t_c(tC, 0, 0, RMID, P64), t_c(tA, 0, 0, RMID, P64),
                        1.0, t_c(tC, 0, 0, RMID, P64), Alu.mult, Alu.add)  # pool(hf)
                    # hf^2 into tA[64:], then poolsq: y-pass tB[64:], x-pass tA[64:]
                    nc.scalar.activation(t_c(tA, parts=P64), fa_c(0, 0, RIN, P64),
                                         Act.Square)  # hf^2
                    nc.vector.tensor_tensor(t_c(tB, parts=P64),
                                            t_c(tA, 0, -1, RIN, P64),
                                            t_c(tA, 0, +1, RIN, P64), Alu.add)
                    nc.vector.scalar_tensor_tensor(
                        t_c(tB, parts=P64), t_c(tA, 0, 0, RIN, P64),
                        1.0, t_c(tB, parts=P64), Alu.mult, Alu.add)
                    nc.vector.tensor_tensor(
                        t_c(tA, 0, 0, RMID, P64), t_c(tB, +1, 0, RMID, P64),
                        t_c(tB, -1, 0, RMID, P64), Alu.add)
                    nc.vector.scalar_tensor_tensor(
                        t_c(tA, 0, 0, RMID, P64), t_c(tB, 0, 0, RMID, P64),
                        1.0, t_c(tA, 0, 0, RMID, P64), Alu.mult, Alu.add)  # poolsq
                    # pool(hf)^2/81 -> tC (Square with scale 1/9)
                    nc.scalar.activation(t_c(tC, 0, 0, RMID, P64),
                                         t_c(tC, 0, 0, RMID, P64), Act.Square,
                                         scale=1.0 / 9.0)
                    # vdiff = poolsq/9 - pool^2/81 -> tC[64:128]
                    nc.vector.scalar_tensor_tensor(
                        t_c(tC, 0, 0, RMID, P64), t_c(tA, 0, 0, RMID, P64),
                        1.0 / 9.0, t_c(tC, 0, 0, RMID, P64),
                        Alu.mult, Alu.subtract)
                    # diff^2 -> tC[0:64]
                    nc.vector.tensor_scalar(
                        t_c(tC, 0, 0, RMID, P0_64), fa_c(0, 0, RMID, P0_64),
                        elft[:, 0:1], None, Alu.subtract)
                    nc.scalar.activation(t_c(tC, 0, 0, RMID, P0_64),
                                         t_c(tC, 0, 0, RMID, P0_64), Act.Square)
                    fnd = fence_mm(tC)
                    for c in range(HC0, HC1):
                        stat_mm(G_ASQ, tC, c, vec[:, 2:4], ncol=2, fence=fnd)

                    # ---- cost dots: f . f_shift for DIRS[0..3] (bf16,
                    # ping-pong over two tiles so DVE runs ahead of LDWs) ----
                    if T <= 8:  # ping-pong tile fits only at T=8 slice width
                        tE16 = cp.tile([C, CHR * SLICE_W + 8], dtb, tag="tE16")
                    else:
                        tE16 = tD16
                    for i in range(4):
                        dtile = (tD16, tE16)[i % 2]
                        dx, dy = DIRS[i]
                        nc.vector.tensor_tensor(
                            t_c(dtile, 0, 0, RMID), fa_c(0, 0, RMID),
                            fa_c(dx, dy, RMID), Alu.mult)
                        fni = fence_mm(dtile, vt=vecb)
                        for c in range(FWs):
                            stat_mm(G_DOT + i, dtile, c, vecb[:, 1:2], fence=fni)

                    # ---- MLP: h = relu(w1.T f + b1) ; wl = w2.T h ----
                    # (two 64-row halves so hrel is half-size)
                    HR = CR // 2
                    for hh in range(2):
                        row0 = TOFF + (x0 + hh * HR + 1) * SLICE_W
                        total = HR * SLICE_W
                        off = 0
                        while off < total:
                            n = min(512, total - off)
                            psh = psbf.tile([32, 512], dt, tag="psh")
                            mmh = nc.tensor.matmul(
                                psh[:, 0:n], w1t[:],
                                fap(fA[:], row0 + off, [[1, n]]),
                                start=True, stop=True)
                            _adh(mmh.ins, f_w1, sync=False, reason="w1 ldw fence")
                            nc.scalar.activation(
                                fap(hrel[:], off, [[1, n]]), psh[:, 0:n], Act.Relu,
                                bias=b1t[:, 0:1])
                            off += n
                        fnh = fence_mm(hrel, parts=(0, 32), flat=True, vt=vec)
                        for c in range(HC0, HC1):
                            stat_mm(G_WL, hrel, c, w2t[:], parts=(0, 32),
                                    halo_off=0, fence=fnh,
                                    rows=(hh * HR, hh * HR + HR))

                # ---------------- evacuate psums to field tile ----------------
                nc.scalar.copy(fap(flds, G_NALL * FXX, [[1, FXX]]),
                               fap(ps_all, G_NALL * FXX, [[1, FXX]]))
                nc.scalar.copy(fap(flds, G_DOT * FXX, [[1, 4 * FXX]]),
                               fap(ps_all, G_DOT * FXX, [[1, 4 * FXX]]))
                nc.scalar.copy(fap(flds, 0, [[1, FXX]]),
                               fap(ps_all, 0, [[1, FXX]]))
                nc.scalar.copy(fap(flds, G_ASQ * FXX, [[1, 3 * FXX]]),
                               fap(ps_all, G_ASQ * FXX, [[1, 3 * FXX]]))

                # ---------------- field-level row shifts (PE) ----------------
                # ps_sh groups: 0 nall(x-1) | 1 nall(x+1) | 2..4 dot_k(x+1)
                su_m = fap(shmt[:], 0 * HP, [[1, HP]])
                useam_m = fap(shmt[:], 1 * HP, [[1, HP]])
                sd_m = fap(shmt[:], 2 * HP, [[1, HP]])
                dseam_m = fap(shmt[:], 3 * HP, [[1, HP]])
                ident_m = fap(shmt[:], 4 * HP, [[1, HP]])

                def shift_mm(ps_t, dst_off, n, stat_ap, src_off, start, stop):
                    mm = nc.tensor.matmul(
                        fap(ps_t, dst_off, [[1, n]]), stat_ap,
                        fap(flds, src_off, [[1, n]]), start=start, stop=stop)
                    _adh(mm.ins, f_shm, sync=False, reason="shm ldw fence")

                # round A: nall(x-1) via sd (+ block1 seam from block0), and
                # nall(x+1) via su (+ block0 seam from block1)
                ps_sh = psp.tile([HP, 3 * FXX], dt, tag="pssh")
                shift_mm(ps_sh, 0, FXX, sd_m, G_NALL * FXX, True, False)
                shift_mm(ps_sh, FW, FW, dseam_m, G_NALL * FXX, False, True)
                shift_mm(ps_sh, FXX, FXX, su_m, G_NALL * FXX, True, False)
                shift_mm(ps_sh, FXX, FW, useam_m, G_NALL * FXX + FW, False, True)
                nc.scalar.copy(fap(flds2, 0, [[1, 2 * FXX]]),
                               fap(ps_sh, 0, [[1, 2 * FXX]]))
                # round B: dot_k(x+1), k=0..2, stored REVERSED (2|1|0) so the
                # shifted e-slots 0..2 read at a uniform stride
                # NOTE: a start=True matmul lazily re-zeroes its whole PSUM
                # bank for subsequent start=False writes, so each group's seam
                # must IMMEDIATELY follow its main (before the next start=True).
                ps_sh2 = psp.tile([HP, 3 * FXX], dt, tag="pssh")
                for k in range(3):
                    shift_mm(ps_sh2, (2 - k) * FXX, FXX, su_m,
                             (G_DOT + k) * FXX, True, False)
                    shift_mm(ps_sh2, (2 - k) * FXX, FW, useam_m,
                             (G_DOT + k) * FXX + FW, False, True)
                nc.scalar.copy(fap(flds2, 2 * FXX, [[1, 3 * FXX]]), ps_sh2[:])

                # ---------------- rs / rsh ----------------
                rs_t = sp.tile([HP, FXX], dt, tag="rs")
                rsh_t = sp.tile([HP, 2 * FXX], dt, tag="rsh")  # rs(x-1) | rs(x+1)
                nc.scalar.activation(rs_t[:], fap(flds, G_NALL * FXX, [[1, FXX]]),
                                     Act.Sqrt)
                nc.vector.tensor_scalar_max(rs_t[:], rs_t[:], 1e-12)
                nc.vector.reciprocal(rs_t[:], rs_t[:])
                nc.scalar.activation(rsh_t[:], fap(flds2, 0, [[1, 2 * FXX]]),
                                     Act.Sqrt)
                nc.vector.tensor_scalar_max(rsh_t[:], rsh_t[:], 1e-12)
                nc.vector.reciprocal(rsh_t[:], rsh_t[:])

                # ---------------- e_t assembly ----------------
                # e_t slot s holds the additive BF candidate cost for group
                # (g, bdir) with s = g*3 + bdir; derivation: e[s](x,y) =
                # cost_d((x,y) - dvec), dvec = (-dxu, 1-bdir), d = index(dvec).
                # Unshifted slots: 3<-cost_3, 6<-cost_0, 7<-cost_1, 8<-cost_2;
                # shifted: 0<-cost_2(x+1,y-1), 1<-cost_1(x+1,y), 2<-cost_0(x+1,y+1),
                # 5<-cost_3(x,y+1); slot 4 = 0 (self). Shifted slots double as
                # output mirror channels cost_5..7 and cost_4.
                e_t = pp.tile([HP, 9 * YLEN], dt, tag="e")
                nc.vector.memset(e_t[:], BIGF)
                nc.vector.memset(fap(e_t[:], 4 * YLEN, [[1, YLEN]]), 0.0)

                def ewin(slot):
                    return fap(e_t[:], slot * YLEN + 1, [[YB, XH], [1, HELD]])

                def fwin(t, goff, dy=0):
                    return fap(t[:], goff * FXX + 1 + dy, [[FW, XH], [1, HELD]])

                def mwin(g):
                    return fap(mskt[:], g * FXX + 1, [[FW, XH], [1, HELD]])

                # batched assembly: each spec covers `ng` e-slots in one
                # 4-op chain via a leading slot-group AP dim
                # (dst_base/dst_stride on e_t; P/Q/R/M = (tile, base, stride))
                def bwin(t, base, stride, ng):
                    return fap(t[:], base, [[stride, ng], [FW, XH], [1, HELD]])

                bspecs = [
                    # slots 6,7,8: cost_0..2 = 1 - dot_d*rs*rsh_m[y+d-1]
                    (6 * YLEN, YLEN, 3,
                     (flds, G_DOT * FXX + 1, FXX), (rsh_t, 0, 1),
                     (rs_t, 1, 0), (mskt, 0 * FXX + 1, FXX)),
                    # slots 0,1,2: shifted variants from reversed dotsh
                    (0, YLEN, 3,
                     (flds2, 2 * FXX, FXX + 1), (rsh_t, FXX, 1),
                     (rs_t, 1, 0), (mskt, 4 * FXX + 1, FXX)),
                    # slots 3,5: cost_3 and its y-shift
                    (3 * YLEN, 2 * YLEN, 2,
                     (flds, (G_DOT + 3) * FXX + 1, 1), (rs_t, 0, 2),
                     (rs_t, 1, 0), (mskt, 3 * FXX + 1, 4 * FXX)),
                ]
                for dbase, dstride, ng, P, Q, R, M in bspecs:
                    dst = fap(e_t[:], dbase + 1, [[dstride, ng], [YB, XH],
                                                  [1, HELD]])
                    nc.vector.tensor_tensor(dst, bwin(*P, ng), bwin(*Q, ng),
                                            Alu.mult)
                    nc.vector.tensor_tensor(dst, dst, bwin(*R, ng), Alu.mult)
                    nc.vector.tensor_scalar(dst, dst, -1.0, 1.0, Alu.mult, Alu.add)
                    nc.vector.tensor_tensor(dst, dst, bwin(*M, ng), Alu.max)

                # ---------------- heuristic assembly ----------------
                om_f = sp.tile([HP, FXX], dt, tag="om")
                nc.scalar.activation(om_f[:], fap(flds, G_WL * FXX, [[1, FXX]]),
                                     Act.Sigmoid, bias=hsct[:, 4:5])
                absb_f = sp.tile([HP, FXX], dt, tag="absb")
                nc.vector.tensor_scalar_max(absb_f[:],
                                            fap(flds, G_ASQ * FXX, [[1, FXX]]), 0.0)
                nc.scalar.activation(absb_f[:], absb_f[:], Act.Sqrt)
                scat_f = sp.tile([HP, FXX], dt, tag="scat")
                nc.vector.tensor_scalar(scat_f[:],
                                        fap(flds, G_VAR * FXX, [[1, FXX]]),
                                        hsct[:, 3:4], -1.0,
                                        Alu.subtract, Alu.mult)
                h1 = sp.tile([HP, FXX], dt, tag="h1")
                h2 = sp.tile([HP, FXX], dt, tag="h2")
                heur_f = sp.tile([HP, FXX], dt, tag="heur")
                nc.vector.tensor_tensor(h1[:], om_f[:], scat_f[:], Alu.mult)
                nc.vector.tensor_tensor(h2[:], om_f[:], absb_f[:], Alu.mult)
                nc.vector.tensor_tensor(h2[:], absb_f[:], h2[:], Alu.subtract)
                nc.vector.tensor_scalar(heur_f[:],
                                        fap(flds, G_GEO * FXX, [[1, FXX]]),
                                        hsct[:, 0:1], None, Alu.mult)
                nc.vector.scalar_tensor_tensor(h1[:], h1[:], hsct[:, 1:2], heur_f[:],
                                               Alu.mult, Alu.add)
                nc.vector.scalar_tensor_tensor(h1[:], h2[:], hsct[:, 2:3], h1[:],
                                               Alu.mult, Alu.add)
                nc.vector.tensor_scalar_max(heur_f[:], h1[:], 0.0)

                # ---------------- output slab (channels 0..8) ----------------
                slab = sp.tile([HP, BAND * XH * 10], dt, tag="slab")
                CO0 = T + 1  # first owned col in field/slot index

                def to_slab(src_ap, ch):
                    dst = fap(slab[:], ch, [[XH * 10, BAND], [10, XH]])
                    nc.vector.tensor_copy(dst, src_ap)

                to_slab(fap(heur_f[:], CO0, [[1, BAND], [FW, XH]]), 0)
                # cost channels (1..8) from e-slots, batched by uniform stride:
                # ch 1,2,3 <- slots 6,7,8; ch 4,5 <- slots 3,5; ch 6,7,8 <- 0,1,2
                for ch0, slot0, sstride, ng in ((1, 6, YLEN, 3), (4, 3, 2 * YLEN, 2),
                                                (6, 0, YLEN, 3)):
                    src_ap = fap(e_t[:], slot0 * YLEN + CO0,
                                 [[sstride, ng], [1, BAND], [YB, XH]])
                    dst_ap = fap(slab[:], ch0, [[1, ng], [XH * 10, BAND], [10, XH]])
                    nc.vector.tensor_copy(dst_ap, src_ap)

                # ------------- Bellman-Ford (PE shifts + PSUM-direct DVE) ------
                tmp_t = pp.tile([HP, 16 * YLEN], dt, tag="tmp")
                bufs = []
                for i in range(2):
                    dbi = pp.tile([HP, YLEN], dt, tag=f"dbuf{i}")
                    bufs.append(dbi)
                contrib = sp.tile([HP, 4 * T], dt, tag="contrib")

                for b in bufs:
                    nc.vector.memset(b[:], BIGF)
                nc.sync.dma_start(bufs[0][:], dinit_in[:])

                if NC > 1:
                    pid = nc.sync.partition_id()
                    jm_off = ((pid + NC - 1) & (NC - 1)) * (HP * 4 * T)
                    jp_off = ((pid + 1) & (NC - 1)) * (HP * 4 * T)

                no_coll = bool(cfg.get("no_coll"))
                wavef = bool(cfg.get("wavefront"))
                bf_tree = bool(cfg.get("bf_tree"))
                f32r = bool(cfg.get("fp32r"))
                rr = (lambda a: a.bitcast(mybir.dt.float32r)) if f32r else (lambda a: a)
                for k in range(K):
                    if wavef and k <= HP - 2:
                        y0, YI = 1, YB - 2
                    else:
                        y0, YI = 1, YLEN - 2
                    cur, nxt = bufs[k % 2], bufs[(k + 1) % 2]
                    if (NC > 1 and k > 0 and k % T == 0 and not no_coll
                            and not (wavef and k < BAND - T)):
                        # ---- halo exchange on cur ----
                        src = fap(cur[:], 1 + T,
                                  [[YB, 2], [BAND - T, 2], [1, T]])
                        nc.vector.tensor_copy(
                            fap(contrib[:], 0, [[2 * T, 2], [T, 2], [1, T]]), src)
                        nc.sync.dma_start(bounce_in[:], contrib[:])
                        nc.gpsimd.collective_compute(
                            "AllGather", mybir.AluOpType.bypass,
                            ins=[bounce_in[:]], outs=[bounce_out[:]],
                            replica_groups=[list(range(NC))])
                        lsrc = AP(bounce_out[:].tensor, jm_off + T,
                                  [[4 * T, HP], [2 * T, 2], [1, T]])
                        nc.sync.dma_start(
                            fap(cur[:], 1, [[YB, 2], [1, T]]), lsrc)
                        rsrc = AP(bounce_out[:].tensor, jp_off + 0,
                                  [[4 * T, HP], [2 * T, 2], [1, T]])
                        nc.sync.dma_start(
                            fap(cur[:], 1 + HELD - T, [[YB, 2], [1, T]]), rsrc)

                    # ---- row shifts into one PSUM tile: [du | ds] ----
                    WL = y0 + YI + 1  # matmul width (block 0 only in wavefront)
                    pboth = psbf.tile([HP, 2 * YLEN], dt, tag="pboth")
                    dcur = fap(cur[:], 0, [[1, WL]])
                    m1 = nc.tensor.matmul(fap(pboth[:], 0, [[1, WL]]), su_m,
                                          dcur, start=True, stop=False)
                    m2 = nc.tensor.matmul(fap(pboth[:], 0, [[1, YB]]), useam_m,
                                          fap(cur[:], YB, [[1, YB]]),
                                          start=False, stop=True)
                    m3 = nc.tensor.matmul(fap(pboth[:], YLEN, [[1, WL]]), sd_m,
                                          dcur, start=True, stop=(WL < YLEN))
                    mms = [m1, m2, m3]
                    if WL == YLEN:
                        m4 = nc.tensor.matmul(fap(pboth[:], YLEN + YB, [[1, YB]]),
                                              dseam_m, fap(cur[:], 0, [[1, YB]]),
                                              start=False, stop=True)
                        mms.append(m4)
                    if k == 0:
                        for m in mms:
                            _adh(m.ins, f_shm, sync=False, reason="shm ldw fence")

                    if bf_tree:
                        # ---- group-major adds (contiguous runs) + pairwise
                        # tree-min: all DVE ops stream YI-length segments ----
                        nc.vector.tensor_tensor(
                            fap(tmp_t[:], y0, [[6 * YLEN, 2], [YLEN, 3], [1, YI]]),
                            fap(pboth[:], y0 - 1, [[YLEN, 2], [1, 3], [1, YI]]),
                            fap(e_t[:], y0, [[6 * YLEN, 2], [YLEN, 3], [1, YI]]),
                            Alu.add)
                        nc.vector.tensor_tensor(
                            fap(tmp_t[:], 3 * YLEN + y0, [[YLEN, 3], [1, YI]]),
                            fap(cur[:], y0 - 1, [[1, 3], [1, YI]]),
                            fap(e_t[:], 3 * YLEN + y0, [[YLEN, 3], [1, YI]]),
                            Alu.add)
                        t_ = lambda g, n=1: fap(tmp_t[:], g * YLEN + y0,
                                                [[YLEN, n], [1, YI]])
                        nc.vector.tensor_tensor(t_(0, 4), t_(0, 4), t_(4, 4),
                                                Alu.min)
                        nc.vector.tensor_tensor(t_(0, 2), t_(0, 2), t_(2, 2),
                                                Alu.min)
                        nc.vector.tensor_tensor(t_(0), t_(0), t_(1), Alu.min)
                        nc.vector.tensor_tensor(fap(nxt[:], y0, [[1, YI]]),
                                                t_(0), t_(8), Alu.min)
                    else:
                        # ---- 2 adds + segmented 9-way reduce-min ----
                        nc.vector.tensor_tensor(
                            fap(tmp_t[:], 3 + 16 * y0, [[1, 3], [16, YI]]),
                            fap(cur[:], y0 - 1, [[1, 3], [1, YI]]),
                            fap(e_t[:], 3 * YLEN + y0, [[YLEN, 3], [1, YI]]),
                            Alu.add)
                        nc.vector.tensor_tensor(
                            fap(tmp_t[:], 16 * y0, [[6, 2], [1, 3], [16, YI]]),
                            fap(pboth[:], y0 - 1, [[YLEN, 2], [1, 3], [1, YI]]),
                            fap(e_t[:], y0, [[6 * YLEN, 2], [YLEN, 3], [1, YI]]),
                            Alu.add)
                        nc.vector.tensor_reduce(
                            fap(nxt[:], y0, [[1, YI]]),
                            fap(tmp_t[:], 16 * y0, [[16, YI], [1, 9]]),
                            axis=mybir.AxisListType.X, op=Alu.min)

                # ---- dist -> slab channel 9 ----
                fin = bufs[K % 2]
                src = fap(fin[:], 1 + T, [[YB, 2], [1, BAND]])
                dst = fap(slab[:], 9, [[10, XH], [XH * 10, BAND]])
                nc.vector.tensor_copy(dst, src)

                nc.sync.dma_start(out_t[:], slab[:])

            for _rep in range(cfg.get("REPS", 1)):
                one_pass()

    nc.compile()
    return nc


# ---------------------------------------------------------------- host ------

def softplus32(x):
    x = np.float32(x)
    return F32(np.log1p(np.exp(np.float64(x))))


def host_prepare(cfg, features, delta, gamma, beta, w1, b1, w2, b2,
                 start_node, end_node):
    H, W, C, NC, T = (cfg[k] for k in ("H", "W", "C", "NC", "T"))
    HP, BAND, HELD, SLICE_W, FW, XP, YB, XH = (
        cfg[k] for k in ("HP", "BAND", "HELD", "SLICE_W", "FW", "XP", "YB", "XH"))
    TOFF = cfg["TOFF"]
    features = np.asarray(features, F32)
    w1 = np.asarray(w1, F32).reshape(C, 32)
    b1 = np.asarray(b1, F32).reshape(32, 1)
    w2 = np.asarray(w2, F32).reshape(32, 1)
    b2 = F32(np.asarray(b2).reshape(()))
    sx_, sy_ = [int(v) for v in np.asarray(start_node).ravel()]
    ex_, ey_ = [int(v) for v in np.asarray(end_node).ravel()]

    d_soft, g_soft, b_soft = softplus32(delta), softplus32(gamma), softplus32(beta)

    # info_goal_hf = var_hf at end node (3x3 window, zero pad, count_include_pad)
    hf = features[:, :, C // 2:]
    x0e, x1e = max(0, ex_ - 1), min(H, ex_ + 2)
    y0e, y1e = max(0, ey_ - 1), min(W, ey_ + 2)
    win = hf[x0e:x1e, y0e:y1e, :].astype(F32)
    s1 = win.sum(axis=(0, 1), dtype=F32) / F32(9.0)
    s2 = (win * win).sum(axis=(0, 1), dtype=F32) / F32(9.0)
    info_goal = F32((s2 - s1 * s1).sum(dtype=F32))
    elf = features[ex_, ey_, :C // 2].astype(F32).reshape(64, 1)

    # shift matrices
    su = np.zeros((HP, HP), F32)
    sd = np.zeros((HP, HP), F32)
    for i in range(HP - 1):
        su[i + 1, i] = 1.0   # du[i] = d[i+1]
        sd[i, i + 1] = 1.0   # ds[i] = d[i-1]
    useam = np.zeros((HP, HP), F32)
    useam[0, HP - 1] = 1.0   # du[HP-1] += d_blk1[0]
    dseam = np.zeros((HP, HP), F32)
    dseam[HP - 1, 0] = 1.0   # ds[0] += d_blk0[HP-1]
    ident = np.eye(HP, dtype=F32)
    shm = np.concatenate([su, useam, sd, dseam, ident], axis=1)

    hsc = np.zeros((HP, 8), F32)
    hsc[:, 0] = d_soft
    hsc[:, 1] = g_soft
    hsc[:, 2] = b_soft
    hsc[:, 3] = info_goal
    hsc[:, 4] = b2

    in_maps = []
    for j in range(NC):
        SC0 = j * BAND - (T + 2)
        fa = np.zeros((XP, SLICE_W, C), F32)
        s0 = max(0, -SC0)
        s1_ = min(SLICE_W, W - SC0)
        fa[1:H + 1, s0:s1_, :] = features[:, SC0 + s0:SC0 + s1_, :]
        fa_flat = np.ascontiguousarray(fa.transpose(2, 0, 1).reshape(C, -1))
        fa_pad = np.zeros((C, cfg["FA_FREE"] + 8), F32)
        fa_pad[:, TOFF:TOFF + cfg["FA_FREE"]] = fa_flat

        # masks: 8 groups, block-major [g][xh][c]
        # groups 0..3: unshifted cost_d; 4..7: shifted slots 0,1,2,5
        msk = np.zeros((HP, 8, XH, FW), F32)
        xs = (np.arange(HP)[:, None, None] +
              HP * np.arange(XH)[None, :, None])          # [HP, XH, 1]
        gc = (j * BAND - (T + 1) +
              np.arange(FW)[None, None, :])               # [1, 1, FW]
        for i in range(4):
            dx, dy = DIRS[i]
            invalid = ((gc < 0) | (gc >= W) | (gc + dy < 0) | (gc + dy >= W) |
                       (xs + dx < 0) | (xs + dx >= H))
            msk[:, i] = np.where(invalid, BIGF, 0.0)
        shift_specs = [(1, -1), (1, 0), (1, 1), (0, 1)]  # slots 0,1,2,5
        for gi, (sx, sy) in enumerate(shift_specs):
            invalid = ((xs + sx < 0) | (xs + sx >= H) |
                       (gc + sy < 0) | (gc + sy >= W) | (gc < 0) | (gc >= W))
            msk[:, 4 + gi] = np.where(invalid, BIGF, 0.0)
        msk = msk.reshape(HP, -1)

        YBc = cfg["YB"]
        dinit = np.full((HP, cfg["YLEN"]), BIGF, F32)
        jloc = sy_ - (j * BAND - T)
        if 0 <= jloc < HELD:
            dinit[sx_ % HP, YBc * (sx_ // HP) + 1 + jloc] = 0.0

        in_maps.append({
            "fa": fa_pad, "w1": w1, "b1": b1, "w2": w2, "elf": elf,
            "hsc": hsc, "msk": msk.astype(F32), "dinit": dinit, "shm": shm,
        })
    return in_maps


def host_assemble(cfg, results):
    H, W, NC = cfg["H"], cfg["W"], cfg["NC"]
    HP, BAND, XH = cfg["HP"], cfg["BAND"], cfg["XH"]
    out = np.zeros((H, W, 10), F32)
    for j in range(NC):
        slab = results[j]["out"].reshape(HP, BAND, XH, 10)
        blk = slab.transpose(2, 0, 1, 3).reshape(H, BAND, 10)
        out[:, j * BAND:(j + 1) * BAND, :] = blk
    return out


_PROG_CACHE = {}


def get_program(cfg):
    key = tuple(sorted((k, v) for k, v in cfg.items()))
    if key not in _PROG_CACHE:
        _PROG_CACHE[key] = build_program(cfg)
    return _PROG_CACHE[key]


def kernel(**inputs):
    cfg = make_cfg()
    if int(np.asarray(inputs["start_node"]).ravel()[0]) == 0:
        cfg["wavefront"] = True
    nc = get_program(cfg)
    in_maps = host_prepare(cfg, **inputs)
    from concourse.bass_utils import run_bass_kernel_spmd
    res = run_bass_kernel_spmd(nc, in_maps, core_ids=list(range(cfg["NC"])))
    return host_assemble(cfg, res.results)
